# revision 1
# baseline (speedup 1.0000x reference)
"""Trainium2 Bass kernel for a 2-layer LayerNorm-LSTM (nn_CustomLSTM).

Reference semantics:
  x: [B=512, S=512, IN=118], two stacked LSTM layers (H1=256, H2=128),
  each followed by LayerNorm over features; returns final h2 [B, H2].

Sharding: data-parallel over batch across 8 NeuronCores (64 rows/core),
weights replicated.  Each core runs the full 512-step scan.

Matmuls run in fp32 (walrus lowers each to 2 half-speed passes; measured
429 ns for N=512 vs 224 ns fp16), replacing the previous fp16 hi/lo
3-matmul splits: 1.57x less PE time at full fp32 accuracy, and all the
hi/lo split bookkeeping (identL matmuls, lo-part DVE ops, fp16 copies)
disappears from the serial chain.

Software pipelining: body(t) emits layer-1 of step t interleaved with
layer-2 of step t-1, so the whole layer-2 chain (gates, LN2, transpose)
hides under layer-1's serial phase of the next step.  The critical cycle
is only: z1 h-matmuls -> gates1 -> c1/h1 -> LN1 -> transpose -> copy.

LN affines (gamma/beta) are folded into consumer weights host-side;
layer-1 bias rides as a ones-row in the x chunk; layer-2 bias as a
2-row fp16 hi/lo matmul.  LN rstd uses the previous step's value as a
Newton seed (3 damped iterations on DVE), as in the original kernel.
"""

import numpy as np

B, S, IN = 512, 512, 118
H1, H2 = 256, 128
NCORES = 8
BM = B // NCORES          # 64 batch rows per core
EPS = 1e-5

_CACHE = {}


def _build_program(T=S, n_newton=3, dbg=None):
    import concourse.bass as bass
    import concourse.bacc as bacc
    import concourse.tile as tile
    from concourse import mybir
    from concourse.masks import make_identity

    f16 = mybir.dt.float16
    f32 = mybir.dt.float32
    Alu = mybir.AluOpType
    Act = mybir.ActivationFunctionType

    nc = bacc.Bacc("TRN2", target_bir_lowering=False)

    KIN = IN + 1   # x rows + ones row (bias fold)

    # DRAM I/O ------------------------------------------------------------
    x_d = nc.declare_dram_parameter("x_aug", [KIN, T, BM], f32, isOutput=False)
    wd = {}
    for name, k, n in (("w1c0", KIN, 4 * H1), ("w1c1", 128, 4 * H1),
                       ("w1c2", 128, 4 * H1), ("w2c0", 128, 4 * H2),
                       ("w2c1", 128, 4 * H2), ("w2c2", 128, 4 * H2)):
        wd[name] = nc.declare_dram_parameter(name, [k, n], f32, isOutput=False)
    b2_d = nc.declare_dram_parameter("b2rows", [2, 4 * H2], f16, isOutput=False)
    ones_d = nc.declare_dram_parameter("ones2", [2, BM], f16, isOutput=False)
    zeros_d = nc.declare_dram_parameter("zerosT", [128, 3 * BM], f32, isOutput=False)
    out_d = nc.declare_dram_parameter("h2_out", [BM, H2], f32, isOutput=True)

    with tile.TileContext(nc) as tc:
        consts = tc.alloc_tile_pool(name="consts", bufs=1)
        gates = tc.alloc_tile_pool(name="gates", bufs=2)
        small = tc.alloc_tile_pool(name="small", bufs=3)
        carry = tc.alloc_tile_pool(name="carry", bufs=2)
        psz1 = tc.alloc_tile_pool(name="psz1", bufs=2, space="PSUM")
        psz2 = tc.alloc_tile_pool(name="psz2", bufs=2, space="PSUM")
        psum_t = tc.alloc_tile_pool(name="psum_t", bufs=2, space="PSUM")

        # --- constants / weights into SBUF --------------------------------
        x_aug = consts.tile([KIN, T, BM], f32)
        nc.sync.dma_start(out=x_aug[:], in_=x_d[:])
        wsb = {}
        for name, d in wd.items():
            k, n = d.shape
            wsb[name] = consts.tile([k, n], f32, name=f"sb_{name}")
            nc.sync.dma_start(out=wsb[name][:], in_=d[:])
        b2rows = consts.tile([2, 4 * H2], f16)
        nc.sync.dma_start(out=b2rows[:], in_=b2_d[:])
        ones2 = consts.tile([2, BM], f16)
        nc.sync.dma_start(out=ones2[:], in_=ones_d[:])

        ident = consts.tile([BM, BM], f32)
        make_identity(nc, ident)
        epst = consts.tile([BM, 1], f32)
        nc.vector.memset(epst, EPS)
        seedlo = consts.tile([BM, 1], f32)
        nc.vector.memset(seedlo, 3.4)

        # --- carries -------------------------------------------------------
        # transposed normalized states: cols [0:64]=h1a, [64:128]=h1b,
        # [128:192]=h2 (h2 chunk lags one step behind h1 chunks)
        hT = carry.tile([128, 3 * BM], f32, tag="hT")
        nc.sync.dma_start(out=hT[:], in_=zeros_d[:])
        c1 = carry.tile([BM, H1], f32, tag="c1")
        c2 = carry.tile([BM, H2], f32, tag="c2")
        nc.vector.memset(c1, 0.0)
        nc.vector.memset(c2, 0.0)
        r1 = r2 = None
        hn2_prev = None

        from concourse.dve_ops import RECIPROCAL_APPROX_NR

        poke_state = {}

        def ham_poke(src):
            """Tiny regular matmul keyed off an LN-chain tile: keeps the PE
            activity window alive during the serial LN phase so HAM doesn't
            re-throttle the clock to 1.2 GHz."""
            pk = poke_state.get("tile")
            if pk is None:
                return
            nc.tensor.matmul(pk, src, ident[:, 0:8], start=True, stop=True)

        def newton_rsqrt(u, var, prefix, rp, niter, out_tag, damp=True,
                         poke=True):
            """rstd = rsqrt(var+eps), Newton from seed rp, clamped output.
            Iteration r' = (k0 - (k1*u*r)*r)*r: one TT plus one
            RECIPROCAL_APPROX_NR custom-DVE op.  First iteration folds a
            0.94 seed pre-damp into its constants so the seed-ratio range
            [0.82, 1.23] maps to [0.77, 1.15], contracted by 3 iters."""
            nc.vector.tensor_scalar(out=u, in0=var, scalar1=epst, scalar2=0.5,
                                    op0=Alu.add, op1=Alu.mult)
            if damp:
                ud = small.tile([BM, 1], f32, tag=f"{prefix}ud", name=f"{prefix}ud")
                nc.vector.tensor_scalar(out=ud, in0=var, scalar1=epst,
                                        scalar2=0.5 * 0.94 ** 3,
                                        op0=Alu.add, op1=Alu.mult)
            r_cur = rp
            for it in range(niter):
                first = it == 0 and damp
                k0 = 1.5 * 0.94 if first else 1.5
                uk = ud if first else u
                q = small.tile([BM, 1], f32, tag=f"{prefix}q", name=f"{prefix}q")
                r_nxt = small.tile([BM, 1], f32, tag=f"{prefix}r", name=f"{prefix}r")
                nc.vector.tensor_tensor(out=q, in0=uk, in1=r_cur, op=Alu.mult)
                nc.vector._custom_dve(RECIPROCAL_APPROX_NR, out=r_nxt,
                                      in0=q, in1=r_cur, s0=k0)
                r_cur = r_nxt
                if poke and niter <= 4:
                    ham_poke(r_cur)
            rg = carry.tile([BM, 1], f32, tag=out_tag, name=f"rg_{out_tag}")
            nc.vector.tensor_scalar(out=rg, in0=r_cur, scalar1=3.0,
                                    scalar2=21.0, op0=Alu.max, op1=Alu.min)
            return rg

        def emit_z1_x(t):
            """x-part matmuls for step t (no recurrent deps) -> new z1 tile."""
            z1 = psz1.tile([BM, 4 * H1], f32, tag="z1", name="z1")
            xt = x_aug[:, t, :]
            nc.tensor.matmul(z1[:, 0:512], xt, wsb["w1c0"][:, 0:512],
                             start=True, stop=False)
            nc.tensor.matmul(z1[:, 512:1024], xt, wsb["w1c0"][:, 512:1024],
                             start=True, stop=False)
            return z1

        # -- prologue: step 0's x-part; zero hn2(-1)
        z1_cur = emit_z1_x(0)
        hn2_init = small.tile([BM, H2], f32, tag="hn2", name="hn2_init")
        nc.vector.memset(hn2_init, 0.0)
        hn2_prev = hn2_init

        # Body(t) emits layer-1 of step t interleaved with layer-2 of step
        # t-1, in per-engine order matching expected runnable times, so the
        # L2 chain never blocks L1's critical ops in an engine FIFO.
        for t in range(T):
            last = t == T - 1
            hT_n = carry.tile([128, 3 * BM], f32, tag="hT", name="hT_n")
            pst = psum_t.tile([128, 3 * BM + 8], f32, tag="pst", name="pst")
            poke_state["tile"] = pst[0:1, 3 * BM:3 * BM + 8]

            # PE: layer-1 h-part for step t (the critical matmuls)
            z1 = z1_cur
            h1a, h1b = hT[:, 0:BM], hT[:, BM:2 * BM]
            nc.tensor.matmul(z1[:, 0:512], h1a, wsb["w1c1"][:, 0:512],
                             start=False, stop=False)
            nc.tensor.matmul(z1[:, 0:512], h1b, wsb["w1c2"][:, 0:512],
                             start=False, stop=True)
            nc.tensor.matmul(z1[:, 512:1024], h1a, wsb["w1c1"][:, 512:1024],
                             start=False, stop=False)
            nc.tensor.matmul(z1[:, 512:1024], h1b, wsb["w1c2"][:, 512:1024],
                             start=False, stop=True)

            # PE: transpose hn2(t-2) -> h2 stationary chunk for z2(t-1).
            # The copy MUST be emitted before the z2 matmul that reads it
            # (Tile infers deps from program order); it rides on Vector,
            # whose queue is empty at body start.
            if t > 0:
                nc.tensor.transpose(pst[:, 2 * BM:3 * BM], hn2_prev, ident)
                nc.vector.tensor_copy(hT_n[:, 2 * BM:3 * BM],
                                      pst[:, 2 * BM:3 * BM])

            # PE: z2(t-1) = bias + h2(t-2) part + h1(t-1) parts
            if t > 0:
                z2 = psz2.tile([BM, 4 * H2], f32, tag="z2", name="z2")
                nc.tensor.matmul(z2[:], ones2, b2rows, start=True, stop=False)
                nc.tensor.matmul(z2[:], hT_n[:, 2 * BM:3 * BM], wsb["w2c2"],
                                 start=False, stop=False)
                nc.tensor.matmul(z2[:], h1a, wsb["w2c0"], start=False,
                                 stop=False)
                nc.tensor.matmul(z2[:], h1b, wsb["w2c1"], start=False,
                                 stop=True)

            # PE: pre-emit next step's x-part (PE food later in the serial
            # phase, after the z2 block)
            if not last:
                z1_nxt = emit_z1_x(t + 1)

            # ScalarE: layer-1 gates, split so the c-chain starts as
            # soon as the first z1 N-chunk [f, g] lands
            # (z layout per gate: f=[0:256], g=[256:512], i=[512:768],
            #  o=[768:1024])
            gg1 = gates.tile([BM, 4 * H1], f32, tag="gg1", name="gg1")
            nc.scalar.activation(out=gg1[:, 0:256], in_=z1[:, 0:256],
                                 func=Act.Sigmoid)
            nc.scalar.activation(out=gg1[:, 256:512], in_=z1[:, 256:512],
                                 func=Act.Tanh)
            nc.scalar.activation(out=gg1[:, 512:768], in_=z1[:, 512:768],
                                 func=Act.Sigmoid)
            nc.scalar.activation(out=gg1[:, 768:1024], in_=z1[:, 768:1024],
                                 func=Act.Sigmoid)

            # layer-1 c/h chain
            fc = small.tile([BM, H1], f32, tag="fc1", name="fc1")
            ig = small.tile([BM, H1], f32, tag="ig1", name="ig1")
            c1n = carry.tile([BM, H1], f32, tag="c1", name="c1n")
            nc.gpsimd.tensor_tensor(out=fc, in0=gg1[:, 0:H1], in1=c1,
                                    op=Alu.mult)
            nc.vector.tensor_tensor(out=ig, in0=gg1[:, 2 * H1:3 * H1],
                                    in1=gg1[:, H1:2 * H1], op=Alu.mult)
            nc.vector.tensor_tensor(out=c1n, in0=fc, in1=ig, op=Alu.add)
            c1 = c1n
            tc1 = small.tile([BM, H1], f32, tag="tc1", name="tc1")
            nc.scalar.activation(out=tc1, in_=c1, func=Act.Tanh)
            h1 = small.tile([BM, H1], f32, tag="h1", name="h1")
            nc.vector.tensor_tensor(out=h1, in0=gg1[:, 3 * H1:4 * H1],
                                    in1=tc1, op=Alu.mult)

            # layer-2 gates (ScalarE, after tc1 in queue; z2 ready by then)
            if t > 0:
                gg2 = gates.tile([BM, 4 * H2], f32, tag="gg2", name="gg2")
                nc.scalar.activation(out=gg2[:, 0:H2], in_=z2[:, 0:H2],
                                     func=Act.Sigmoid)
                nc.scalar.activation(out=gg2[:, H2:2 * H2],
                                     in_=z2[:, H2:2 * H2], func=Act.Tanh)
                nc.scalar.activation(out=gg2[:, 2 * H2:4 * H2],
                                     in_=z2[:, 2 * H2:4 * H2],
                                     func=Act.Sigmoid)

            # LN1 stats (Vector, critical)
            st = small.tile([BM, 6], f32, tag="st1", name="st1")
            mv = small.tile([BM, 2], f32, tag="mv1", name="mv1")
            nc.vector.bn_stats(out=st, in_=h1)
            nc.vector.bn_aggr(out=mv, in_=st)

            # layer-2 c-chain front (Vector slots between aggr1 and NR1)
            if t > 0:
                fc2 = small.tile([BM, H2], f32, tag="fc2", name="fc2")
                ig2 = small.tile([BM, H2], f32, tag="ig2", name="ig2")
                c2n = carry.tile([BM, H2], f32, tag="c2", name="c2n")
                nc.gpsimd.tensor_tensor(out=fc2, in0=gg2[:, 0:H2], in1=c2,
                                        op=Alu.mult)
                nc.vector.tensor_tensor(out=ig2, in0=gg2[:, 2 * H2:3 * H2],
                                        in1=gg2[:, H2:2 * H2], op=Alu.mult)
                nc.vector.tensor_tensor(out=c2n, in0=fc2, in1=ig2, op=Alu.add)
                c2 = c2n
                tc2 = small.tile([BM, H2], f32, tag="tc2", name="tc2")
                nc.scalar.activation(out=tc2, in_=c2, func=Act.Tanh)

            # LN1 rsqrt + normalize + transpose (critical tail)
            u1 = small.tile([BM, 1], f32, tag="u1", name="u1")
            boot1 = t < 6
            r1 = newton_rsqrt(u1, mv[:, 1:2], "n1",
                              seedlo if boot1 else r1,
                              10 if boot1 else n_newton, "r1", damp=not boot1)
            hn1 = small.tile([BM, H1], f32, tag="hn1", name="hn1")
            nc.vector.tensor_scalar(
                out=hn1, in0=h1, scalar1=mv[:, 0:1], scalar2=r1,
                op0=Alu.subtract, op1=Alu.mult)
            nc.tensor.transpose(pst[:, 0:BM], hn1[:, 0:128], ident)
            nc.tensor.transpose(pst[:, BM:2 * BM], hn1[:, 128:256], ident)
            nc.scalar.copy(out=hT_n[:, 0:2 * BM], in_=pst[:, 0:2 * BM])

            # layer-2 tail: h2, LN2 (all slack — completes during next body)
            if t > 0:
                h2 = small.tile([BM, H2], f32, tag="h2", name="h2")
                nc.vector.tensor_tensor(out=h2, in0=gg2[:, 3 * H2:4 * H2],
                                        in1=tc2, op=Alu.mult)
                st2 = small.tile([BM, 6], f32, tag="st2", name="st2")
                mv2 = small.tile([BM, 2], f32, tag="mv2", name="mv2")
                nc.vector.bn_stats(out=st2, in_=h2)
                nc.vector.bn_aggr(out=mv2, in_=st2)
                u2 = small.tile([BM, 1], f32, tag="u2", name="u2")
                boot2 = t - 1 < 6
                r2 = newton_rsqrt(u2, mv2[:, 1:2], "n2",
                                  seedlo if boot2 else r2,
                                  10 if boot2 else n_newton, "r2",
                                  damp=not boot2, poke=False)
                hn2 = small.tile([BM, H2], f32, tag="hn2", name="hn2")
                nc.vector.tensor_scalar(
                    out=hn2, in0=h2, scalar1=mv2[:, 0:1], scalar2=r2,
                    op0=Alu.subtract, op1=Alu.mult)
                hn2_prev = hn2

            hT = hT_n
            z1_cur = z1_nxt if not last else None

        # ---------------- epilogue: layer 2 of step T-1 ----------------
        # hT holds h1(T-1) transposed chunks and h2(T-2); hn2_prev is
        # hn2(T-2)... no: hn2_prev = hn2(T-2) was consumed in body T-1.
        # After body T-1, hn2_prev = hn2(T-2)?? -> it was reassigned in
        # body T-1 (t>0 branch) to hn2(T-2+1)=hn2(T-1-1). Transpose it now.
        pstf = psum_t.tile([128, 3 * BM + 8], f32, tag="pst", name="pstf")
        nc.tensor.transpose(pstf[:, 2 * BM:3 * BM], hn2_prev, ident)
        hTf = carry.tile([128, 3 * BM], f32, tag="hT", name="hTf")
        nc.scalar.copy(out=hTf[:, 2 * BM:3 * BM], in_=pstf[:, 2 * BM:3 * BM])
        z2f = psz2.tile([BM, 4 * H2], f32, tag="z2", name="z2f")
        nc.tensor.matmul(z2f[:], ones2, b2rows, start=True, stop=False)
        nc.tensor.matmul(z2f[:], hTf[:, 2 * BM:3 * BM], wsb["w2c2"],
                         start=False, stop=False)
        nc.tensor.matmul(z2f[:], hT[:, 0:BM], wsb["w2c0"], start=False,
                         stop=False)
        nc.tensor.matmul(z2f[:], hT[:, BM:2 * BM], wsb["w2c1"], start=False,
                         stop=True)
        gg2f = gates.tile([BM, 4 * H2], f32, tag="gg2", name="gg2f")
        nc.scalar.activation(out=gg2f[:, 0:H2], in_=z2f[:, 0:H2],
                             func=Act.Sigmoid)
        nc.scalar.activation(out=gg2f[:, H2:2 * H2], in_=z2f[:, H2:2 * H2],
                             func=Act.Tanh)
        nc.scalar.activation(out=gg2f[:, 2 * H2:4 * H2],
                             in_=z2f[:, 2 * H2:4 * H2], func=Act.Sigmoid)
        fc2f = small.tile([BM, H2], f32, tag="fc2", name="fc2f")
        ig2f = small.tile([BM, H2], f32, tag="ig2", name="ig2f")
        c2f = carry.tile([BM, H2], f32, tag="c2", name="c2f")
        nc.gpsimd.tensor_tensor(out=fc2f, in0=gg2f[:, 0:H2], in1=c2,
                                op=Alu.mult)
        nc.vector.tensor_tensor(out=ig2f, in0=gg2f[:, 2 * H2:3 * H2],
                                in1=gg2f[:, H2:2 * H2], op=Alu.mult)
        nc.vector.tensor_tensor(out=c2f, in0=fc2f, in1=ig2f, op=Alu.add)
        tc2f = small.tile([BM, H2], f32, tag="tc2", name="tc2f")
        nc.scalar.activation(out=tc2f, in_=c2f, func=Act.Tanh)
        h2f = small.tile([BM, H2], f32, tag="h2", name="h2f")
        nc.vector.tensor_tensor(out=h2f, in0=gg2f[:, 3 * H2:4 * H2],
                                in1=tc2f, op=Alu.mult)
        st2f = small.tile([BM, 6], f32, tag="st2", name="st2f")
        mv2f = small.tile([BM, 2], f32, tag="mv2", name="mv2f")
        nc.vector.bn_stats(out=st2f, in_=h2f)
        nc.vector.bn_aggr(out=mv2f, in_=st2f)
        u2f = small.tile([BM, 1], f32, tag="u2", name="u2f")
        bootf = T - 1 < 6
        r2 = newton_rsqrt(u2f, mv2f[:, 1:2], "n2",
                          seedlo if bootf else r2,
                          10 if bootf else n_newton, "r2", damp=not bootf,
                          poke=False)
        hn2f = small.tile([BM, H2], f32, tag="hn2", name="hn2f")
        nc.vector.tensor_scalar(
            out=hn2f, in0=h2f, scalar1=mv2f[:, 0:1], scalar2=r2,
            op0=Alu.subtract, op1=Alu.mult)
        nc.sync.dma_start(out=out_d[:], in_=hn2f)

        for p in (psum_t, psz2, psz1, carry, small, gates, consts):
            p.release()

    if not nc.is_finalized():
        nc.finalize()
    return nc


def _prep_host_inputs(x, Wf1, Wi1, Wg1, Wo1, bf1, bi1, bg1, bo1,
                      Wf2, Wi2, Wg2, Wo2, bf2, bi2, bg2, bo2,
                      gamma1, beta1, gamma2, beta2, T=S):
    """Fold LN affines into consumer weights; build fp32 chunks."""
    f = np.float32
    W1 = np.concatenate([Wf1, Wg1, Wi1, Wo1], axis=0).astype(f)   # [1024, 374]
    b1 = np.concatenate([bf1, bg1, bi1, bo1], axis=0).astype(f)
    W2 = np.concatenate([Wf2, Wg2, Wi2, Wo2], axis=0).astype(f)   # [512, 384]
    b2 = np.concatenate([bf2, bg2, bi2, bo2], axis=0).astype(f)
    g1v, b1v = gamma1.astype(f), beta1.astype(f)
    g2v, b2v = gamma2.astype(f), beta2.astype(f)

    W1x = W1[:, :IN]
    W1h = W1[:, IN:]
    b1p = b1 + W1h @ b1v
    W1hp = W1h * g1v[None, :]
    W2h1 = W2[:, :H1]
    W2h2 = W2[:, H1:]
    b2p = b2 + W2h1 @ b1v + W2h2 @ b2v
    W2h1p = W2h1 * g1v[None, :]
    W2h2p = W2h2 * g2v[None, :]

    warrs = {
        "w1c0": np.ascontiguousarray(
            np.concatenate([W1x.T, b1p[None, :]], axis=0), dtype=f),
        "w1c1": np.ascontiguousarray(W1hp.T[0:128], dtype=f),
        "w1c2": np.ascontiguousarray(W1hp.T[128:256], dtype=f),
        "w2c0": np.ascontiguousarray(W2h1p.T[0:128], dtype=f),
        "w2c1": np.ascontiguousarray(W2h1p.T[128:256], dtype=f),
        "w2c2": np.ascontiguousarray(W2h2p.T, dtype=f),
    }

    b2hi = b2p.astype(np.float16)
    b2lo = (b2p - b2hi.astype(f)).astype(np.float16)
    b2rows = np.stack([b2hi, b2lo])                                # [2, 512]

    in_maps = []
    for i in range(NCORES):
        xs = np.asarray(x[i * BM:(i + 1) * BM, :T, :], dtype=f)    # [64, T, 118]
        xT = np.transpose(xs, (2, 1, 0))                           # [118, T, 64]
        x_aug = np.concatenate(
            [xT, np.ones((1, T, BM), dtype=f)], axis=0)            # [119, T, 64]
        in_maps.append(dict(
            x_aug=np.ascontiguousarray(x_aug),
            b2rows=b2rows,
            ones2=np.stack([np.ones(BM, np.float16),
                            np.ones(BM, np.float16)]),
            zerosT=np.zeros((128, 3 * BM), dtype=f),
            **warrs,
        ))
    return in_maps


def kernel(**inputs):
    from concourse.bass_utils import run_bass_kernel_spmd

    T = S
    if "prog" not in _CACHE:
        _CACHE["prog"] = _build_program(T)
    nc = _CACHE["prog"]

    in_maps = _prep_host_inputs(**inputs, T=T)
    res = run_bass_kernel_spmd(nc, in_maps, list(range(NCORES)))
    parts = [np.asarray(res.results[i]["h2_out"]) for i in range(NCORES)]
    h2 = np.concatenate(parts, axis=0)                             # [512, 128]
    g2v = np.asarray(inputs["gamma2"], dtype=np.float32)
    b2v = np.asarray(inputs["beta2"], dtype=np.float32)
    return (h2 * g2v[None, :] + b2v[None, :]).astype(np.float32)


if __name__ == "__main__":
    print("building program...")
    _build_program(T=4)
    print("ok")

